# revision 22
# baseline (speedup 1.0000x reference)
"""GreedySampler kernel for 8 Trainium2 NeuronCores.

The reference gathers 200 "last token" rows of hidden_states, computes
logits against the 50257x4096 embedding, and argmaxes over vocab
(softmax/log are monotonic). Cost is dominated by streaming the
embedding matrix: memory-bound. Tensor-parallel over vocab: each core
streams a 6144-column fp8 shard (25.2MB) once and computes fp8
DoubleRow logits; the host shortlists columns within DELTA of each
row's max and rescores them exactly in f64, so fp8 only nominates
candidates — it never decides the winner. The ~2% vocab tail left by
128-alignment is scored exactly on the host (one dgemm, off the device
critical path).

Device-side structure (vs the first working version, 96.3us):

  * W is pre-packed on the host into the exact SBUF image the PE wants,
    so every W load is a [128, 4KB-contiguous-per-partition] DMA
    (previously 512B gather lines) on the two HWDGE rings (sync/scalar,
    alternating). hst load + mid-stream logit stores ride SWDGE
    (gpsimd) so a store's sem-wait never head-of-line-blocks W loads;
    only the final store uses HWDGE for its shorter fixed latency.
  * Per-128-column vocab sub-block W DMAs start the PE after ~2us and
    give fine-grained load/compute overlap.
  * DoubleRowSwInterleave weight layout (host-interleaved) lets the HW
    read the stationary operand as one contiguous 256B/partition
    stream; the plain DoubleRow weight load (two non-contiguous
    128-column passes, ~+72% LDWEIGHTS time) was the likely critical
    path of the 96.3us version (800 x ~120ns = its entire runtime).
  * 40 warm-up matmuls on zeroed data during the DMA startup window
    hold PE activity so the HAM clock gate sits at 8/8 (2.4 GHz) when
    the first real matmul issues.
  * Logit stores batch 5 sub-blocks into [128, 1000B] lines (>=512B,
    no read-modify-write penalty) in a partition-major dram layout the
    host un-permutes; store groups taper at the end so the pipeline
    drains through a small final store.

Notes:
  * This walrus build rejects instructions carrying more than one sync
    wait, so after Tile scheduling we split excess waits onto nop
    instructions inserted just before the offender on the same engine
    queue (in-order execution keeps the semantics identical).
  * SwInterleave stationary contract (validated against HW): the
    flattened [p, 2, 128] AP view is pair-interleaved with reversed
    columns - block[2j + t] = W[col 127-j, k-row t].
"""

import numpy as np
import ml_dtypes

import concourse.bass as bass
import concourse.mybir as mybir
import concourse.tile as tile
from concourse.vector_clock import ScopedClock
from concourse.bass_utils import run_bass_kernel_spmd

P = 128
N_CORES = 8
W_SCALE = 32.0
DELTA = 2.0 * W_SCALE  # candidate margin in scaled-logit units

FP8 = mybir.dt.float8e4
F32 = mybir.dt.float32

USE_SWINTERLEAVE = True

_drain_patched = False


def _patch_tile_drain():
    """Split the tail Drain's sync waits (>1 rejected by this walrus)."""
    global _drain_patched
    if _drain_patched:
        return

    def _drain_and_barrier(self, tick_clock, wait_clock):
        nc = self.nc
        drain_inst = nc.sync.drain()
        wait_clock.add_sem_waits(
            drain_inst.ins, ScopedClock({None: tick_clock.global_clock})
        )
        si = drain_inst.ins.sync_info
        if si is not None and si.on_wait and len(si.on_wait) > 1:
            extra = list(si.on_wait[1:])
            del si.on_wait[1:]
            name2sem = {
                getattr(s, "name", None): s
                for s in self.sems.allocated().values()
            }
            for w in extra:
                nc.sync.wait_ge(name2sem[w.ant_name], w.wait_value)
        nc.all_engine_barrier()
        popped = nc._tile_sem_poison_stack.pop()
        assert popped is self._sem_poison
        nc.clear_and_free_semaphores(list(self.sems.allocated().values()))
        nc.all_engine_barrier()

    tile.TileContext._drain_and_barrier = _drain_and_barrier
    _drain_patched = True


def _split_excess_waits(nc, limit=1):
    """Move all but `limit` sync waits of every instruction onto nops
    inserted immediately before it on the same engine queue."""
    fn = nc.m.functions[0]
    for bb in fn.blocks:
        if not any(
            getattr(i, "sync_info", None) is not None
            and i.sync_info.on_wait
            and len(i.sync_info.on_wait) > limit
            for i in bb.instructions
        ):
            continue
        cur = nc.cur_bb.bb if hasattr(nc.cur_bb, "bb") else nc.cur_bb
        new_insts = []
        for inst in bb.instructions:
            si = getattr(inst, "sync_info", None)
            if si is not None and si.on_wait and len(si.on_wait) > limit:
                extra = list(si.on_wait[:-limit])
                del si.on_wait[: len(si.on_wait) - limit]
                for w in extra:
                    nop = nc.engines[inst.engine].nop(nofuse=True).ins
                    popped = cur.instructions.pop()  # nop() self-appended
                    assert popped is nop
                    nop.sync_info = mybir.SyncInfo(on_wait=[w], on_update=[])
                    new_insts.append(nop)
            new_insts.append(inst)
        bb.instructions[:] = new_insts
    return nc


def build_nc(D, J, VS, store_group=None, swinterleave=USE_SWINTERLEAVE,
             w_bufs=16, ps_bufs=7, out_bufs=3, warmup_mms=40):
    """One core: logits for VS vocab columns x J jobs, fp8 in/out, fp32
    accumulation. W arrives pre-packed as [P, NSUB, KK, 256] where each
    256-byte block is the stationary operand for (sub, kk)."""
    _patch_tile_drain()
    KK = D // (2 * P)
    NSUB = VS // P
    if store_group is None:
        # Groups of 5 sub-blocks, tapering at the end so the final
        # stores are small and clear the pipeline quickly.
        groups, rem = [], NSUB
        while rem > 8:
            groups.append(5)
            rem -= 5
        groups += [rem - 4, 2, 2] if rem > 4 else [rem]
    elif isinstance(store_group, int):
        assert NSUB % store_group == 0
        groups = [store_group] * (NSUB // store_group)
    else:
        groups = list(store_group)
        assert sum(groups) == NSUB
    perf_mode = (
        mybir.MatmulPerfMode.DoubleRowSwInterleave
        if swinterleave
        else mybir.MatmulPerfMode.DoubleRow
    )

    nc = bass.Bass()
    hst = nc.dram_tensor("hst", [P, KK, 2, J], FP8, kind="ExternalInput")
    wt = nc.dram_tensor("wt", [P, NSUB, KK, 2 * P], FP8, kind="ExternalInput")
    # partition-major logits: out[p, s*J + j] = logits[s*128 + p, j]
    out = nc.dram_tensor("out", [P, NSUB * J], FP8, kind="ExternalOutput")

    with tile.TileContext(nc) as tc:
        with (
            tc.tile_pool(name="hs", bufs=1) as hs_pool,
            tc.tile_pool(name="w", bufs=w_bufs) as w_pool,
            tc.tile_pool(name="out", bufs=out_bufs) as out_pool,
            tc.tile_pool(name="ps", bufs=ps_bufs, space=bass.MemorySpace.PSUM) as ps_pool,
            tc.tile_pool(name="wu", bufs=1, space=bass.MemorySpace.PSUM) as wu_pool,
            tc.tile_pool(name="wub", bufs=1) as wub_pool,
        ):
            hst_sb = hs_pool.tile([P, KK, 2, J], FP8)
            nc.gpsimd.dma_start(hst_sb[:], hst[:])

            if warmup_mms:
                # Dummy matmuls on zeroed data fill the DMA startup window
                # with PE activity so the HAM clock gate is already at
                # 8/8 (2.4 GHz) when the first real matmul issues.
                wu = wub_pool.tile([P, 16], FP8, name="wu")
                nc.vector.memset(wu[:], 0)
                wu_ps = wu_pool.tile([P, 512], F32, name="wu_ps")
                for _ in range(warmup_mms):
                    nc.tensor.matmul(
                        wu_ps[:8, :16],
                        wu[:, :8],
                        wu[:, :],
                        start=True,
                        stop=True,
                    )

            wq = [nc.sync, nc.scalar]
            s = 0
            for g, SG in enumerate(groups):
                g0 = s
                ot = out_pool.tile([P, SG, J], FP8, name="ot")
                for s5 in range(SG):
                    w_sb = w_pool.tile([P, KK, 2 * P], FP8, name="w_sb")
                    wq[s % 2].dma_start(w_sb[:], wt[:, s])
                    ps = ps_pool.tile([P, 512], F32, name="ps")
                    for kk in range(KK):
                        nc.tensor.matmul(
                            ps[:, :J],
                            w_sb[:, kk, :].rearrange("p (t m) -> p t m", t=2),
                            hst_sb[:, kk, :, :],
                            start=(kk == 0),
                            stop=(kk == KK - 1),
                            perf_mode=perf_mode,
                        )
                    nc.vector.tensor_copy(ot[:, s5, :], ps[:, :J])
                    s += 1
                # Mid-stream stores ride SWDGE (Pool sequencer is idle) so a
                # store's sem-wait never head-of-line-blocks W loads queued
                # on sync/scalar; the final store (no W loads behind it)
                # uses HWDGE for its shorter fixed latency.
                seng = wq[g % 2] if g == len(groups) - 1 else nc.gpsimd
                seng.dma_start(
                    out[:, g0 * J : s * J].rearrange("p (s j) -> p s j", s=SG),
                    ot[:],
                )

    _split_excess_waits(nc, limit=1)
    return nc


def pack_weights(Wq_core, swinterleave=USE_SWINTERLEAVE):
    """[VS, D] fp8 -> [P, NSUB, KK, 256] stationary image.

    DoubleRow:      block[t*128 + m] = Wq[s*128 + m,     kk*256 + t*128 + p]
    SwInterleave:   block[2*j + t]   = Wq[s*128 + 127-j, kk*256 + t*128 + p]
    """
    VS, D = Wq_core.shape
    KK = D // 256
    NSUB = VS // P
    A = np.ascontiguousarray(
        Wq_core.reshape(NSUB, P, KK, 2, P)
    )  # [s, m, kk, t, p]
    if swinterleave:
        Ar = A[:, ::-1]  # j = 127 - m
        out = Ar.transpose(4, 0, 2, 1, 3)  # [p, s, kk, j, t]
    else:
        out = A.transpose(4, 0, 2, 3, 1)  # [p, s, kk, t, m]
    return np.ascontiguousarray(out).reshape(P, NSUB, KK, 2 * P)


def pack_hidden(hs_sel_fp8):
    """[J, D] fp8 -> [P, KK, 2, J] with d = kk*256 + t*128 + p."""
    J, D = hs_sel_fp8.shape
    return np.ascontiguousarray(
        hs_sel_fp8.T.reshape(D // 256, 2, P, J).transpose(2, 0, 1, 3)
    )


def _job_indices(fill_tokens_num, num_generation_jobs):
    fill = np.asarray(fill_tokens_num, dtype=np.int64)
    fill_last = np.cumsum(fill) - 1
    total_fill = int(fill.sum())
    gen = total_fill + np.arange(int(num_generation_jobs), dtype=np.int64)
    return np.concatenate([fill_last, gen])


def kernel(hidden_states, embd_weight, fill_tokens_num, num_generation_jobs):
    hs = np.asarray(hidden_states, dtype=np.float32)
    W = np.asarray(embd_weight, dtype=np.float32)
    V, D = W.shape

    idx = _job_indices(fill_tokens_num, num_generation_jobs)
    J = idx.size

    hs_sel = hs[idx]  # [J, D] f32, kept for the exact rescore
    hst_host = pack_hidden(hs_sel.astype(ml_dtypes.float8_e4m3))

    # Device covers an even 128-aligned vocab shard per core; the short
    # tail (~2% of vocab) is scored exactly on the host (one dgemm, off
    # the device critical path).
    VS = (V // (N_CORES * P) - 1) * P  # per-core vocab shard width
    V_dev = VS * N_CORES
    Wq = (W[:V_dev] * W_SCALE).astype(ml_dtypes.float8_e4m3)
    in_maps = [
        {
            "hst": hst_host,
            "wt": pack_weights(Wq[i * VS : (i + 1) * VS]),
        }
        for i in range(N_CORES)
    ]

    nc = build_nc(D, J, VS)
    kernel.last_nc = nc
    kernel.last_in_maps = in_maps
    res = run_bass_kernel_spmd(nc, in_maps, core_ids=list(range(N_CORES)))
    kernel.last_results = res

    # out[p, s*J + j] = logits[s*128 + p, j]; concat cores over vocab.
    logits = np.concatenate(
        [
            res.results[i]["out"]
            .astype(np.float32)
            .reshape(P, VS // P, J)
            .transpose(1, 0, 2)
            .reshape(VS, J)
            for i in range(N_CORES)
        ],
        axis=0,
    ).T
    # Saturation note: max |scaled logit| here is ~287 > TRN e4m3's 240.
    # TRN clamps all overflow to its inf encoding, which ml_dtypes reads
    # as 256, so every saturated column ties the row max (m = 256) and
    # stays a candidate; the shortlist then keeps everything >= 192
    # scaled, and any true argmax in a row with saturation reads >= 220
    # (248 - fp8 err), so it is always shortlisted. Requires
    # DELTA >= (256 - 248) + 2*err ~ 35 scaled; DELTA = 64. NaN inputs
    # (true NaN only, not range overflow) map to +inf to stay candidates.
    logits = np.where(np.isnan(logits), np.inf, logits)

    # Columns within DELTA of each row's max, rescored exactly in f64.
    m = logits.max(axis=1, keepdims=True)
    rows, cols = np.nonzero(logits >= m - DELTA)
    exact = np.einsum(
        "ij,ij->i", hs_sel[rows].astype(np.float64), W[cols].astype(np.float64)
    )
    ids = np.zeros(J, dtype=np.int64)
    best = np.full(J, -np.inf)
    for r, c, s in zip(rows, cols, exact):
        if s > best[r]:
            best[r] = s
            ids[r] = c

    # Exact host logits for the vocab tail the device didn't cover.
    tail = hs_sel.astype(np.float64) @ W[V_dev:].astype(np.float64).T
    tail_best = tail.max(axis=1)
    tail_ids = V_dev + tail.argmax(axis=1)
    take = tail_best > best
    ids[take] = tail_ids[take]
    return ids.astype(np.int32)


# revision 29
# speedup vs baseline: 1.0027x; 1.0027x over previous
"""GreedySampler kernel for 8 Trainium2 NeuronCores.

The reference gathers 200 "last token" rows of hidden_states, computes
logits against the 50257x4096 embedding, and argmaxes over vocab
(softmax/log are monotonic). Cost is dominated by streaming the
embedding matrix: memory-bound. Tensor-parallel over vocab: each core
streams a 6144-column fp8 shard (25.2MB) once and computes fp8
DoubleRow logits; the host shortlists columns within DELTA of each
row's max and rescores them exactly in f64, so fp8 only nominates
candidates — it never decides the winner. The ~2% vocab tail left by
128-alignment is scored exactly on the host (one dgemm, off the device
critical path).

Device-side structure (vs the first working version, 96.3us):

  * W is pre-packed on the host into the exact SBUF image the PE wants,
    so every W load is a [128, 4KB-contiguous-per-partition] DMA
    (previously 512B gather lines) on the two HWDGE rings (sync/scalar,
    alternating). hst load + mid-stream logit stores ride SWDGE
    (gpsimd) so a store's sem-wait never head-of-line-blocks W loads;
    only the final store uses HWDGE for its shorter fixed latency.
  * Per-128-column vocab sub-block W DMAs start the PE after ~2us and
    give fine-grained load/compute overlap.
  * DoubleRowSwInterleave weight layout (host-interleaved) lets the HW
    read the stationary operand as one contiguous 256B/partition
    stream; the plain DoubleRow weight load (two non-contiguous
    128-column passes, ~+72% LDWEIGHTS time) was the likely critical
    path of the 96.3us version (800 x ~120ns = its entire runtime).
  * 40 warm-up matmuls on zeroed data during the DMA startup window
    hold PE activity so the HAM clock gate sits at 8/8 (2.4 GHz) when
    the first real matmul issues.
  * Logit stores batch 5 sub-blocks into [128, 1000B] lines (>=512B,
    no read-modify-write penalty) in a partition-major dram layout the
    host un-permutes; store groups taper at the end so the pipeline
    drains through a small final store.

Notes:
  * This walrus build rejects instructions carrying more than one sync
    wait, so after Tile scheduling we split excess waits onto nop
    instructions inserted just before the offender on the same engine
    queue (in-order execution keeps the semantics identical).
  * SwInterleave stationary contract (validated against HW): the
    flattened [p, 2, 128] AP view is pair-interleaved with reversed
    columns - block[2j + t] = W[col 127-j, k-row t].
"""

import numpy as np
import ml_dtypes

import concourse.bass as bass
import concourse.mybir as mybir
import concourse.tile as tile
from concourse.vector_clock import ScopedClock
from concourse.bass_utils import run_bass_kernel_spmd

P = 128
N_CORES = 8
W_SCALE = 32.0
DELTA = 2.0 * W_SCALE  # candidate margin in scaled-logit units

FP8 = mybir.dt.float8e4
F32 = mybir.dt.float32

USE_SWINTERLEAVE = True

_drain_patched = False


def _patch_tile_drain():
    """Split the tail Drain's sync waits (>1 rejected by this walrus)."""
    global _drain_patched
    if _drain_patched:
        return

    def _drain_and_barrier(self, tick_clock, wait_clock):
        nc = self.nc
        drain_inst = nc.sync.drain()
        wait_clock.add_sem_waits(
            drain_inst.ins, ScopedClock({None: tick_clock.global_clock})
        )
        si = drain_inst.ins.sync_info
        if si is not None and si.on_wait and len(si.on_wait) > 1:
            extra = list(si.on_wait[1:])
            del si.on_wait[1:]
            name2sem = {
                getattr(s, "name", None): s
                for s in self.sems.allocated().values()
            }
            for w in extra:
                nc.sync.wait_ge(name2sem[w.ant_name], w.wait_value)
        nc.all_engine_barrier()
        popped = nc._tile_sem_poison_stack.pop()
        assert popped is self._sem_poison
        nc.clear_and_free_semaphores(list(self.sems.allocated().values()))
        nc.all_engine_barrier()

    tile.TileContext._drain_and_barrier = _drain_and_barrier
    _drain_patched = True


def _split_excess_waits(nc, limit=1):
    """Move all but `limit` sync waits of every instruction onto nops
    inserted immediately before it on the same engine queue."""
    fn = nc.m.functions[0]
    for bb in fn.blocks:
        if not any(
            getattr(i, "sync_info", None) is not None
            and i.sync_info.on_wait
            and len(i.sync_info.on_wait) > limit
            for i in bb.instructions
        ):
            continue
        cur = nc.cur_bb.bb if hasattr(nc.cur_bb, "bb") else nc.cur_bb
        new_insts = []
        for inst in bb.instructions:
            si = getattr(inst, "sync_info", None)
            if si is not None and si.on_wait and len(si.on_wait) > limit:
                extra = list(si.on_wait[:-limit])
                del si.on_wait[: len(si.on_wait) - limit]
                for w in extra:
                    nop = nc.engines[inst.engine].nop(nofuse=True).ins
                    popped = cur.instructions.pop()  # nop() self-appended
                    assert popped is nop
                    nop.sync_info = mybir.SyncInfo(on_wait=[w], on_update=[])
                    new_insts.append(nop)
            new_insts.append(inst)
        bb.instructions[:] = new_insts
    return nc


def build_nc(D, J, VS, store_group=None, swinterleave=USE_SWINTERLEAVE,
             w_bufs=16, ps_bufs=7, out_bufs=3, warmup_mms=56,
             split_tail_loads=1):
    """One core: logits for VS vocab columns x J jobs, fp8 in/out, fp32
    accumulation. W arrives pre-packed as [P, NSUB, KK, 256] where each
    256-byte block is the stationary operand for (sub, kk)."""
    _patch_tile_drain()
    KK = D // (2 * P)
    NSUB = VS // P
    if store_group is None:
        # Groups of 5 sub-blocks, tapering at the end so the final
        # stores are small and clear the pipeline quickly.
        groups, rem = [], NSUB
        while rem > 8:
            groups.append(5)
            rem -= 5
        groups += [rem - 4, 2, 2] if rem > 4 else [rem]
    elif isinstance(store_group, int):
        assert NSUB % store_group == 0
        groups = [store_group] * (NSUB // store_group)
    else:
        groups = list(store_group)
        assert sum(groups) == NSUB
    perf_mode = (
        mybir.MatmulPerfMode.DoubleRowSwInterleave
        if swinterleave
        else mybir.MatmulPerfMode.DoubleRow
    )

    nc = bass.Bass()
    hst = nc.dram_tensor("hst", [P, KK, 2, J], FP8, kind="ExternalInput")
    wt = nc.dram_tensor("wt", [P, NSUB, KK, 2 * P], FP8, kind="ExternalInput")
    # partition-major logits: out[p, s*J + j] = logits[s*128 + p, j]
    out = nc.dram_tensor("out", [P, NSUB * J], FP8, kind="ExternalOutput")

    with tile.TileContext(nc) as tc:
        with (
            tc.tile_pool(name="hs", bufs=1) as hs_pool,
            tc.tile_pool(name="w", bufs=w_bufs) as w_pool,
            tc.tile_pool(name="out", bufs=out_bufs) as out_pool,
            tc.tile_pool(name="ps", bufs=ps_bufs, space=bass.MemorySpace.PSUM) as ps_pool,
            tc.tile_pool(name="wu", bufs=1, space=bass.MemorySpace.PSUM) as wu_pool,
            tc.tile_pool(name="wub", bufs=1) as wub_pool,
        ):
            hst_sb = hs_pool.tile([P, KK, 2, J], FP8)
            nc.gpsimd.dma_start(hst_sb[:], hst[:])

            if warmup_mms:
                # Dummy matmuls on zeroed data fill the DMA startup window
                # with PE activity so the HAM clock gate is already at
                # 8/8 (2.4 GHz) when the first real matmul issues.
                wu = wub_pool.tile([P, 16], FP8, name="wu")
                nc.vector.memset(wu[:], 0)
                wu_ps = wu_pool.tile([P, 512], F32, name="wu_ps")
                for _ in range(warmup_mms):
                    nc.tensor.matmul(
                        wu_ps[:8, :16],
                        wu[:, :8],
                        wu[:, :],
                        start=True,
                        stop=True,
                    )

            wq = [nc.sync, nc.scalar]
            s = 0
            for g, SG in enumerate(groups):
                g0 = s
                ot = out_pool.tile([P, SG, J], FP8, name="ot")
                for s5 in range(SG):
                    w_sb = w_pool.tile([P, KK, 2 * P], FP8, name="w_sb")
                    if s >= NSUB - split_tail_loads:
                        # The last loads gate the serial tail: split them
                        # so their matmuls start after half the line.
                        wq[s % 2].dma_start(
                            w_sb[:, : KK // 2], wt[:, s, : KK // 2]
                        )
                        wq[s % 2].dma_start(
                            w_sb[:, KK // 2 :], wt[:, s, KK // 2 :]
                        )
                    else:
                        wq[s % 2].dma_start(w_sb[:], wt[:, s])
                    ps = ps_pool.tile([P, 512], F32, name="ps")
                    for kk in range(KK):
                        nc.tensor.matmul(
                            ps[:, :J],
                            w_sb[:, kk, :].rearrange("p (t m) -> p t m", t=2),
                            hst_sb[:, kk, :, :],
                            start=(kk == 0),
                            stop=(kk == KK - 1),
                            perf_mode=perf_mode,
                        )
                    nc.vector.tensor_copy(ot[:, s5, :], ps[:, :J])
                    s += 1
                # Mid-stream stores ride SWDGE (Pool sequencer is idle) so a
                # store's sem-wait never head-of-line-blocks W loads queued
                # on sync/scalar; the final store (no W loads behind it)
                # uses HWDGE for its shorter fixed latency.
                seng = wq[g % 2] if g == len(groups) - 1 else nc.gpsimd
                seng.dma_start(
                    out[:, g0 * J : s * J].rearrange("p (s j) -> p s j", s=SG),
                    ot[:],
                )

    _split_excess_waits(nc, limit=1)
    return nc


def pack_weights(Wq_core, swinterleave=USE_SWINTERLEAVE):
    """[VS, D] fp8 -> [P, NSUB, KK, 256] stationary image.

    DoubleRow:      block[t*128 + m] = Wq[s*128 + m,     kk*256 + t*128 + p]
    SwInterleave:   block[2*j + t]   = Wq[s*128 + 127-j, kk*256 + t*128 + p]
    """
    VS, D = Wq_core.shape
    KK = D // 256
    NSUB = VS // P
    A = np.ascontiguousarray(
        Wq_core.reshape(NSUB, P, KK, 2, P)
    )  # [s, m, kk, t, p]
    if swinterleave:
        Ar = A[:, ::-1]  # j = 127 - m
        out = Ar.transpose(4, 0, 2, 1, 3)  # [p, s, kk, j, t]
    else:
        out = A.transpose(4, 0, 2, 3, 1)  # [p, s, kk, t, m]
    return np.ascontiguousarray(out).reshape(P, NSUB, KK, 2 * P)


def pack_hidden(hs_sel_fp8):
    """[J, D] fp8 -> [P, KK, 2, J] with d = kk*256 + t*128 + p."""
    J, D = hs_sel_fp8.shape
    return np.ascontiguousarray(
        hs_sel_fp8.T.reshape(D // 256, 2, P, J).transpose(2, 0, 1, 3)
    )


def _job_indices(fill_tokens_num, num_generation_jobs):
    fill = np.asarray(fill_tokens_num, dtype=np.int64)
    fill_last = np.cumsum(fill) - 1
    total_fill = int(fill.sum())
    gen = total_fill + np.arange(int(num_generation_jobs), dtype=np.int64)
    return np.concatenate([fill_last, gen])


def kernel(hidden_states, embd_weight, fill_tokens_num, num_generation_jobs):
    hs = np.asarray(hidden_states, dtype=np.float32)
    W = np.asarray(embd_weight, dtype=np.float32)
    V, D = W.shape

    idx = _job_indices(fill_tokens_num, num_generation_jobs)
    J = idx.size

    hs_sel = hs[idx]  # [J, D] f32, kept for the exact rescore
    hst_host = pack_hidden(hs_sel.astype(ml_dtypes.float8_e4m3))

    # Device covers an even 128-aligned vocab shard per core; the short
    # tail (~2% of vocab) is scored exactly on the host (one dgemm, off
    # the device critical path).
    VS = (V // (N_CORES * P) - 1) * P  # per-core vocab shard width
    V_dev = VS * N_CORES
    Wq = (W[:V_dev] * W_SCALE).astype(ml_dtypes.float8_e4m3)
    in_maps = [
        {
            "hst": hst_host,
            "wt": pack_weights(Wq[i * VS : (i + 1) * VS]),
        }
        for i in range(N_CORES)
    ]

    nc = build_nc(D, J, VS)
    kernel.last_nc = nc
    kernel.last_in_maps = in_maps
    res = run_bass_kernel_spmd(nc, in_maps, core_ids=list(range(N_CORES)))
    kernel.last_results = res

    # out[p, s*J + j] = logits[s*128 + p, j]; concat cores over vocab.
    logits = np.concatenate(
        [
            res.results[i]["out"]
            .astype(np.float32)
            .reshape(P, VS // P, J)
            .transpose(1, 0, 2)
            .reshape(VS, J)
            for i in range(N_CORES)
        ],
        axis=0,
    ).T
    # Saturation note: max |scaled logit| here is ~287 > TRN e4m3's 240.
    # TRN clamps all overflow to its inf encoding, which ml_dtypes reads
    # as 256, so every saturated column ties the row max (m = 256) and
    # stays a candidate; the shortlist then keeps everything >= 192
    # scaled, and any true argmax in a row with saturation reads >= 220
    # (248 - fp8 err), so it is always shortlisted. Requires
    # DELTA >= (256 - 248) + 2*err ~ 35 scaled; DELTA = 64. NaN inputs
    # (true NaN only, not range overflow) map to +inf to stay candidates.
    logits = np.where(np.isnan(logits), np.inf, logits)

    # Columns within DELTA of each row's max, rescored exactly in f64.
    m = logits.max(axis=1, keepdims=True)
    rows, cols = np.nonzero(logits >= m - DELTA)
    exact = np.einsum(
        "ij,ij->i", hs_sel[rows].astype(np.float64), W[cols].astype(np.float64)
    )
    ids = np.zeros(J, dtype=np.int64)
    best = np.full(J, -np.inf)
    for r, c, s in zip(rows, cols, exact):
        if s > best[r]:
            best[r] = s
            ids[r] = c

    # Exact host logits for the vocab tail the device didn't cover.
    tail = hs_sel.astype(np.float64) @ W[V_dev:].astype(np.float64).T
    tail_best = tail.max(axis=1)
    tail_ids = V_dev + tail.argmax(axis=1)
    take = tail_best > best
    ids[take] = tail_ids[take]
    return ids.astype(np.int32)


# revision 30
# speedup vs baseline: 1.0044x; 1.0016x over previous
"""GreedySampler kernel for 8 Trainium2 NeuronCores.

The reference gathers 200 "last token" rows of hidden_states, computes
logits against the 50257x4096 embedding, and argmaxes over vocab
(softmax/log are monotonic). Cost is dominated by streaming the
embedding matrix: memory-bound. Tensor-parallel over vocab: each core
streams a 6144-column fp8 shard (25.2MB) once and computes fp8
DoubleRow logits; the host shortlists columns within DELTA of each
row's max and rescores them exactly in f64, so fp8 only nominates
candidates — it never decides the winner. The ~2% vocab tail left by
128-alignment is scored exactly on the host (one dgemm, off the device
critical path).

Device-side structure (vs the first working version, 96.3us):

  * W is pre-packed on the host into the exact SBUF image the PE wants,
    so every W load is a [128, 4KB-contiguous-per-partition] DMA
    (previously 512B gather lines) on the two HWDGE rings (sync/scalar,
    alternating). hst load + mid-stream logit stores ride SWDGE
    (gpsimd) so a store's sem-wait never head-of-line-blocks W loads;
    only the final store uses HWDGE for its shorter fixed latency.
  * Per-128-column vocab sub-block W DMAs start the PE after ~2us and
    give fine-grained load/compute overlap.
  * DoubleRowSwInterleave weight layout (host-interleaved) lets the HW
    read the stationary operand as one contiguous 256B/partition
    stream; the plain DoubleRow weight load (two non-contiguous
    128-column passes, ~+72% LDWEIGHTS time) was the likely critical
    path of the 96.3us version (800 x ~120ns = its entire runtime).
  * 40 warm-up matmuls on zeroed data during the DMA startup window
    hold PE activity so the HAM clock gate sits at 8/8 (2.4 GHz) when
    the first real matmul issues.
  * Logit stores batch 5 sub-blocks into [128, 1000B] lines (>=512B,
    no read-modify-write penalty) in a partition-major dram layout the
    host un-permutes; store groups taper at the end so the pipeline
    drains through a small final store.

Notes:
  * This walrus build rejects instructions carrying more than one sync
    wait, so after Tile scheduling we split excess waits onto nop
    instructions inserted just before the offender on the same engine
    queue (in-order execution keeps the semantics identical).
  * SwInterleave stationary contract (validated against HW): the
    flattened [p, 2, 128] AP view is pair-interleaved with reversed
    columns - block[2j + t] = W[col 127-j, k-row t].
"""

import numpy as np
import ml_dtypes

import concourse.bass as bass
import concourse.mybir as mybir
import concourse.tile as tile
from concourse.vector_clock import ScopedClock
from concourse.bass_utils import run_bass_kernel_spmd

P = 128
N_CORES = 8
W_SCALE = 32.0
DELTA = 2.0 * W_SCALE  # candidate margin in scaled-logit units

FP8 = mybir.dt.float8e4
F32 = mybir.dt.float32

USE_SWINTERLEAVE = True

_drain_patched = False


def _patch_tile_drain():
    """Split the tail Drain's sync waits (>1 rejected by this walrus)."""
    global _drain_patched
    if _drain_patched:
        return

    def _drain_and_barrier(self, tick_clock, wait_clock):
        nc = self.nc
        drain_inst = nc.sync.drain()
        wait_clock.add_sem_waits(
            drain_inst.ins, ScopedClock({None: tick_clock.global_clock})
        )
        si = drain_inst.ins.sync_info
        if si is not None and si.on_wait and len(si.on_wait) > 1:
            extra = list(si.on_wait[1:])
            del si.on_wait[1:]
            name2sem = {
                getattr(s, "name", None): s
                for s in self.sems.allocated().values()
            }
            for w in extra:
                nc.sync.wait_ge(name2sem[w.ant_name], w.wait_value)
        nc.all_engine_barrier()
        popped = nc._tile_sem_poison_stack.pop()
        assert popped is self._sem_poison
        nc.clear_and_free_semaphores(list(self.sems.allocated().values()))
        nc.all_engine_barrier()

    tile.TileContext._drain_and_barrier = _drain_and_barrier
    _drain_patched = True


def _split_excess_waits(nc, limit=1):
    """Move all but `limit` sync waits of every instruction onto nops
    inserted immediately before it on the same engine queue."""
    fn = nc.m.functions[0]
    for bb in fn.blocks:
        if not any(
            getattr(i, "sync_info", None) is not None
            and i.sync_info.on_wait
            and len(i.sync_info.on_wait) > limit
            for i in bb.instructions
        ):
            continue
        cur = nc.cur_bb.bb if hasattr(nc.cur_bb, "bb") else nc.cur_bb
        new_insts = []
        for inst in bb.instructions:
            si = getattr(inst, "sync_info", None)
            if si is not None and si.on_wait and len(si.on_wait) > limit:
                extra = list(si.on_wait[:-limit])
                del si.on_wait[: len(si.on_wait) - limit]
                for w in extra:
                    nop = nc.engines[inst.engine].nop(nofuse=True).ins
                    popped = cur.instructions.pop()  # nop() self-appended
                    assert popped is nop
                    nop.sync_info = mybir.SyncInfo(on_wait=[w], on_update=[])
                    new_insts.append(nop)
            new_insts.append(inst)
        bb.instructions[:] = new_insts
    return nc


def build_nc(D, J, VS, store_group=None, swinterleave=USE_SWINTERLEAVE,
             w_bufs=16, ps_bufs=7, out_bufs=3, warmup_mms=56,
             split_tail_loads=1):
    """One core: logits for VS vocab columns x J jobs, fp8 in/out, fp32
    accumulation. W arrives pre-packed as [P, NSUB, KK, 256] where each
    256-byte block is the stationary operand for (sub, kk)."""
    _patch_tile_drain()
    KK = D // (2 * P)
    NSUB = VS // P
    if store_group is None:
        # Groups of 5 sub-blocks, tapering at the end so the final
        # stores are small and clear the pipeline quickly.
        groups, rem = [], NSUB
        while rem > 8:
            groups.append(5)
            rem -= 5
        groups += [rem - 4, 3, 1] if rem > 4 else [rem]
    elif isinstance(store_group, int):
        assert NSUB % store_group == 0
        groups = [store_group] * (NSUB // store_group)
    else:
        groups = list(store_group)
        assert sum(groups) == NSUB
    perf_mode = (
        mybir.MatmulPerfMode.DoubleRowSwInterleave
        if swinterleave
        else mybir.MatmulPerfMode.DoubleRow
    )

    nc = bass.Bass()
    hst = nc.dram_tensor("hst", [P, KK, 2, J], FP8, kind="ExternalInput")
    wt = nc.dram_tensor("wt", [P, NSUB, KK, 2 * P], FP8, kind="ExternalInput")
    # partition-major logits: out[p, s*J + j] = logits[s*128 + p, j]
    out = nc.dram_tensor("out", [P, NSUB * J], FP8, kind="ExternalOutput")

    with tile.TileContext(nc) as tc:
        with (
            tc.tile_pool(name="hs", bufs=1) as hs_pool,
            tc.tile_pool(name="w", bufs=w_bufs) as w_pool,
            tc.tile_pool(name="out", bufs=out_bufs) as out_pool,
            tc.tile_pool(name="ps", bufs=ps_bufs, space=bass.MemorySpace.PSUM) as ps_pool,
            tc.tile_pool(name="wu", bufs=1, space=bass.MemorySpace.PSUM) as wu_pool,
            tc.tile_pool(name="wub", bufs=1) as wub_pool,
        ):
            hst_sb = hs_pool.tile([P, KK, 2, J], FP8)
            nc.gpsimd.dma_start(hst_sb[:], hst[:])

            if warmup_mms:
                # Dummy matmuls on zeroed data fill the DMA startup window
                # with PE activity so the HAM clock gate is already at
                # 8/8 (2.4 GHz) when the first real matmul issues.
                wu = wub_pool.tile([P, 16], FP8, name="wu")
                nc.vector.memset(wu[:], 0)
                wu_ps = wu_pool.tile([P, 512], F32, name="wu_ps")
                for _ in range(warmup_mms):
                    nc.tensor.matmul(
                        wu_ps[:8, :16],
                        wu[:, :8],
                        wu[:, :],
                        start=True,
                        stop=True,
                    )

            wq = [nc.sync, nc.scalar]
            s = 0
            for g, SG in enumerate(groups):
                g0 = s
                ot = out_pool.tile([P, SG, J], FP8, name="ot")
                for s5 in range(SG):
                    w_sb = w_pool.tile([P, KK, 2 * P], FP8, name="w_sb")
                    if s >= NSUB - split_tail_loads:
                        # The last loads gate the serial tail: split them
                        # so their matmuls start after half the line.
                        wq[s % 2].dma_start(
                            w_sb[:, : KK // 2], wt[:, s, : KK // 2]
                        )
                        wq[s % 2].dma_start(
                            w_sb[:, KK // 2 :], wt[:, s, KK // 2 :]
                        )
                    else:
                        wq[s % 2].dma_start(w_sb[:], wt[:, s])
                    ps = ps_pool.tile([P, 512], F32, name="ps")
                    for kk in range(KK):
                        nc.tensor.matmul(
                            ps[:, :J],
                            w_sb[:, kk, :].rearrange("p (t m) -> p t m", t=2),
                            hst_sb[:, kk, :, :],
                            start=(kk == 0),
                            stop=(kk == KK - 1),
                            perf_mode=perf_mode,
                        )
                    nc.vector.tensor_copy(ot[:, s5, :], ps[:, :J])
                    s += 1
                # Mid-stream stores ride SWDGE (Pool sequencer is idle) so a
                # store's sem-wait never head-of-line-blocks W loads queued
                # on sync/scalar; the final store (no W loads behind it)
                # uses HWDGE for its shorter fixed latency.
                seng = wq[g % 2] if g == len(groups) - 1 else nc.gpsimd
                seng.dma_start(
                    out[:, g0 * J : s * J].rearrange("p (s j) -> p s j", s=SG),
                    ot[:],
                )

    _split_excess_waits(nc, limit=1)
    return nc


def pack_weights(Wq_core, swinterleave=USE_SWINTERLEAVE):
    """[VS, D] fp8 -> [P, NSUB, KK, 256] stationary image.

    DoubleRow:      block[t*128 + m] = Wq[s*128 + m,     kk*256 + t*128 + p]
    SwInterleave:   block[2*j + t]   = Wq[s*128 + 127-j, kk*256 + t*128 + p]
    """
    VS, D = Wq_core.shape
    KK = D // 256
    NSUB = VS // P
    A = np.ascontiguousarray(
        Wq_core.reshape(NSUB, P, KK, 2, P)
    )  # [s, m, kk, t, p]
    if swinterleave:
        Ar = A[:, ::-1]  # j = 127 - m
        out = Ar.transpose(4, 0, 2, 1, 3)  # [p, s, kk, j, t]
    else:
        out = A.transpose(4, 0, 2, 3, 1)  # [p, s, kk, t, m]
    return np.ascontiguousarray(out).reshape(P, NSUB, KK, 2 * P)


def pack_hidden(hs_sel_fp8):
    """[J, D] fp8 -> [P, KK, 2, J] with d = kk*256 + t*128 + p."""
    J, D = hs_sel_fp8.shape
    return np.ascontiguousarray(
        hs_sel_fp8.T.reshape(D // 256, 2, P, J).transpose(2, 0, 1, 3)
    )


def _job_indices(fill_tokens_num, num_generation_jobs):
    fill = np.asarray(fill_tokens_num, dtype=np.int64)
    fill_last = np.cumsum(fill) - 1
    total_fill = int(fill.sum())
    gen = total_fill + np.arange(int(num_generation_jobs), dtype=np.int64)
    return np.concatenate([fill_last, gen])


def kernel(hidden_states, embd_weight, fill_tokens_num, num_generation_jobs):
    hs = np.asarray(hidden_states, dtype=np.float32)
    W = np.asarray(embd_weight, dtype=np.float32)
    V, D = W.shape

    idx = _job_indices(fill_tokens_num, num_generation_jobs)
    J = idx.size

    hs_sel = hs[idx]  # [J, D] f32, kept for the exact rescore
    hst_host = pack_hidden(hs_sel.astype(ml_dtypes.float8_e4m3))

    # Device covers an even 128-aligned vocab shard per core; the short
    # tail (~2% of vocab) is scored exactly on the host (one dgemm, off
    # the device critical path).
    VS = (V // (N_CORES * P) - 1) * P  # per-core vocab shard width
    V_dev = VS * N_CORES
    Wq = (W[:V_dev] * W_SCALE).astype(ml_dtypes.float8_e4m3)
    in_maps = [
        {
            "hst": hst_host,
            "wt": pack_weights(Wq[i * VS : (i + 1) * VS]),
        }
        for i in range(N_CORES)
    ]

    nc = build_nc(D, J, VS)
    kernel.last_nc = nc
    kernel.last_in_maps = in_maps
    res = run_bass_kernel_spmd(nc, in_maps, core_ids=list(range(N_CORES)))
    kernel.last_results = res

    # out[p, s*J + j] = logits[s*128 + p, j]; concat cores over vocab.
    logits = np.concatenate(
        [
            res.results[i]["out"]
            .astype(np.float32)
            .reshape(P, VS // P, J)
            .transpose(1, 0, 2)
            .reshape(VS, J)
            for i in range(N_CORES)
        ],
        axis=0,
    ).T
    # Saturation note: max |scaled logit| here is ~287 > TRN e4m3's 240.
    # TRN clamps all overflow to its inf encoding, which ml_dtypes reads
    # as 256, so every saturated column ties the row max (m = 256) and
    # stays a candidate; the shortlist then keeps everything >= 192
    # scaled, and any true argmax in a row with saturation reads >= 220
    # (248 - fp8 err), so it is always shortlisted. Requires
    # DELTA >= (256 - 248) + 2*err ~ 35 scaled; DELTA = 64. NaN inputs
    # (true NaN only, not range overflow) map to +inf to stay candidates.
    logits = np.where(np.isnan(logits), np.inf, logits)

    # Columns within DELTA of each row's max, rescored exactly in f64.
    m = logits.max(axis=1, keepdims=True)
    rows, cols = np.nonzero(logits >= m - DELTA)
    exact = np.einsum(
        "ij,ij->i", hs_sel[rows].astype(np.float64), W[cols].astype(np.float64)
    )
    ids = np.zeros(J, dtype=np.int64)
    best = np.full(J, -np.inf)
    for r, c, s in zip(rows, cols, exact):
        if s > best[r]:
            best[r] = s
            ids[r] = c

    # Exact host logits for the vocab tail the device didn't cover.
    tail = hs_sel.astype(np.float64) @ W[V_dev:].astype(np.float64).T
    tail_best = tail.max(axis=1)
    tail_ids = V_dev + tail.argmax(axis=1)
    take = tail_best > best
    ids[take] = tail_ids[take]
    return ids.astype(np.int32)


# revision 32
# speedup vs baseline: 1.0075x; 1.0031x over previous
"""GreedySampler kernel for 8 Trainium2 NeuronCores.

The reference gathers 200 "last token" rows of hidden_states, computes
logits against the 50257x4096 embedding, and argmaxes over vocab
(softmax/log are monotonic). Cost is dominated by streaming the
embedding matrix: memory-bound. Tensor-parallel over vocab: each core
streams a 6144-column fp8 shard (25.2MB) once and computes fp8
DoubleRow logits; the host shortlists columns within DELTA of each
row's max and rescores them exactly in f64, so fp8 only nominates
candidates — it never decides the winner. The ~2% vocab tail left by
128-alignment is scored exactly on the host (one dgemm, off the device
critical path).

Device-side structure (vs the first working version, 96.3us):

  * W is pre-packed on the host into the exact SBUF image the PE wants,
    so every W load is a [128, 4KB-contiguous-per-partition] DMA
    (previously 512B gather lines) on the two HWDGE rings (sync/scalar,
    alternating). hst load + mid-stream logit stores ride SWDGE
    (gpsimd) so a store's sem-wait never head-of-line-blocks W loads;
    only the final store uses HWDGE for its shorter fixed latency.
  * Per-128-column vocab sub-block W DMAs start the PE after ~2us and
    give fine-grained load/compute overlap.
  * DoubleRowSwInterleave weight layout (host-interleaved) lets the HW
    read the stationary operand as one contiguous 256B/partition
    stream; the plain DoubleRow weight load (two non-contiguous
    128-column passes, ~+72% LDWEIGHTS time) was the likely critical
    path of the 96.3us version (800 x ~120ns = its entire runtime).
  * 40 warm-up matmuls on zeroed data during the DMA startup window
    hold PE activity so the HAM clock gate sits at 8/8 (2.4 GHz) when
    the first real matmul issues.
  * Logit stores batch 5 sub-blocks into [128, 1000B] lines (>=512B,
    no read-modify-write penalty) in a partition-major dram layout the
    host un-permutes; store groups taper at the end so the pipeline
    drains through a small final store.

Notes:
  * This walrus build rejects instructions carrying more than one sync
    wait, so after Tile scheduling we split excess waits onto nop
    instructions inserted just before the offender on the same engine
    queue (in-order execution keeps the semantics identical).
  * SwInterleave stationary contract (validated against HW): the
    flattened [p, 2, 128] AP view is pair-interleaved with reversed
    columns - block[2j + t] = W[col 127-j, k-row t].
"""

import numpy as np
import ml_dtypes

import concourse.bass as bass
import concourse.mybir as mybir
import concourse.tile as tile
from concourse.vector_clock import ScopedClock
from concourse.bass_utils import run_bass_kernel_spmd

P = 128
N_CORES = 8
W_SCALE = 32.0
DELTA = 2.0 * W_SCALE  # candidate margin in scaled-logit units

FP8 = mybir.dt.float8e4
F32 = mybir.dt.float32

USE_SWINTERLEAVE = True

_drain_patched = False


def _patch_tile_drain():
    """Split the tail Drain's sync waits (>1 rejected by this walrus)."""
    global _drain_patched
    if _drain_patched:
        return

    def _drain_and_barrier(self, tick_clock, wait_clock):
        nc = self.nc
        drain_inst = nc.sync.drain()
        wait_clock.add_sem_waits(
            drain_inst.ins, ScopedClock({None: tick_clock.global_clock})
        )
        si = drain_inst.ins.sync_info
        if si is not None and si.on_wait and len(si.on_wait) > 1:
            extra = list(si.on_wait[1:])
            del si.on_wait[1:]
            name2sem = {
                getattr(s, "name", None): s
                for s in self.sems.allocated().values()
            }
            for w in extra:
                nc.sync.wait_ge(name2sem[w.ant_name], w.wait_value)
        nc.all_engine_barrier()
        popped = nc._tile_sem_poison_stack.pop()
        assert popped is self._sem_poison
        nc.clear_and_free_semaphores(list(self.sems.allocated().values()))
        nc.all_engine_barrier()

    tile.TileContext._drain_and_barrier = _drain_and_barrier
    _drain_patched = True


def _split_excess_waits(nc, limit=1):
    """Move all but `limit` sync waits of every instruction onto nops
    inserted immediately before it on the same engine queue."""
    fn = nc.m.functions[0]
    for bb in fn.blocks:
        if not any(
            getattr(i, "sync_info", None) is not None
            and i.sync_info.on_wait
            and len(i.sync_info.on_wait) > limit
            for i in bb.instructions
        ):
            continue
        cur = nc.cur_bb.bb if hasattr(nc.cur_bb, "bb") else nc.cur_bb
        new_insts = []
        for inst in bb.instructions:
            si = getattr(inst, "sync_info", None)
            if si is not None and si.on_wait and len(si.on_wait) > limit:
                extra = list(si.on_wait[:-limit])
                del si.on_wait[: len(si.on_wait) - limit]
                for w in extra:
                    nop = nc.engines[inst.engine].nop(nofuse=True).ins
                    popped = cur.instructions.pop()  # nop() self-appended
                    assert popped is nop
                    nop.sync_info = mybir.SyncInfo(on_wait=[w], on_update=[])
                    new_insts.append(nop)
            new_insts.append(inst)
        bb.instructions[:] = new_insts
    return nc


def _strip_const_memsets(nc):
    """Drop the Bass-preamble memsets of the never-read const-* tensors
    (no readers in this kernel, no sync info) so the startup barrier —
    and with it the first W DMA — completes ~0.4us earlier."""
    for bb in nc.m.functions[0].blocks:
        keep = []
        for inst in bb.instructions:
            if type(inst).__name__ == "InstMemset" and "const-" in str(inst):
                si = getattr(inst, "sync_info", None)
                assert si is None or (not si.on_wait and not si.on_update)
                continue
            keep.append(inst)
        bb.instructions[:] = keep
    return nc


def build_nc(D, J, VS, store_group=None, swinterleave=USE_SWINTERLEAVE,
             w_bufs=16, ps_bufs=7, out_bufs=3, warmup_mms=56,
             split_tail_loads=1):
    """One core: logits for VS vocab columns x J jobs, fp8 in/out, fp32
    accumulation. W arrives pre-packed as [P, NSUB, KK, 256] where each
    256-byte block is the stationary operand for (sub, kk)."""
    _patch_tile_drain()
    KK = D // (2 * P)
    NSUB = VS // P
    if store_group is None:
        # Groups of 5 sub-blocks, tapering at the end so the final
        # stores are small and clear the pipeline quickly.
        groups, rem = [], NSUB
        while rem > 8:
            groups.append(5)
            rem -= 5
        groups += [rem - 4, 3, 1] if rem > 4 else [rem]
    elif isinstance(store_group, int):
        assert NSUB % store_group == 0
        groups = [store_group] * (NSUB // store_group)
    else:
        groups = list(store_group)
        assert sum(groups) == NSUB
    perf_mode = (
        mybir.MatmulPerfMode.DoubleRowSwInterleave
        if swinterleave
        else mybir.MatmulPerfMode.DoubleRow
    )

    nc = bass.Bass()
    hst = nc.dram_tensor("hst", [P, KK, 2, J], FP8, kind="ExternalInput")
    wt = nc.dram_tensor("wt", [P, NSUB, KK, 2 * P], FP8, kind="ExternalInput")
    # partition-major logits: out[p, s*J + j] = logits[s*128 + p, j]
    out = nc.dram_tensor("out", [P, NSUB * J], FP8, kind="ExternalOutput")

    with tile.TileContext(nc) as tc:
        with (
            tc.tile_pool(name="hs", bufs=1) as hs_pool,
            tc.tile_pool(name="w", bufs=w_bufs) as w_pool,
            tc.tile_pool(name="out", bufs=out_bufs) as out_pool,
            tc.tile_pool(name="ps", bufs=ps_bufs, space=bass.MemorySpace.PSUM) as ps_pool,
            tc.tile_pool(name="wu", bufs=1, space=bass.MemorySpace.PSUM) as wu_pool,
            tc.tile_pool(name="wub", bufs=1) as wub_pool,
        ):
            hst_sb = hs_pool.tile([P, KK, 2, J], FP8)
            nc.gpsimd.dma_start(hst_sb[:], hst[:])

            if warmup_mms:
                # Dummy matmuls on zeroed data fill the DMA startup window
                # with PE activity so the HAM clock gate is already at
                # 8/8 (2.4 GHz) when the first real matmul issues.
                wu = wub_pool.tile([P, 16], FP8, name="wu")
                nc.vector.memset(wu[:], 0)
                wu_ps = wu_pool.tile([P, 512], F32, name="wu_ps")
                for _ in range(warmup_mms):
                    nc.tensor.matmul(
                        wu_ps[:8, :16],
                        wu[:, :8],
                        wu[:, :],
                        start=True,
                        stop=True,
                    )

            wq = [nc.sync, nc.scalar]
            s = 0
            for g, SG in enumerate(groups):
                g0 = s
                ot = out_pool.tile([P, SG, J], FP8, name="ot")
                for s5 in range(SG):
                    w_sb = w_pool.tile([P, KK, 2 * P], FP8, name="w_sb")
                    if s >= NSUB - split_tail_loads:
                        # The last loads gate the serial tail: split them
                        # so their matmuls start after half the line.
                        wq[s % 2].dma_start(
                            w_sb[:, : KK // 2], wt[:, s, : KK // 2]
                        )
                        wq[s % 2].dma_start(
                            w_sb[:, KK // 2 :], wt[:, s, KK // 2 :]
                        )
                    else:
                        wq[s % 2].dma_start(w_sb[:], wt[:, s])
                    ps = ps_pool.tile([P, 512], F32, name="ps")
                    for kk in range(KK):
                        nc.tensor.matmul(
                            ps[:, :J],
                            w_sb[:, kk, :].rearrange("p (t m) -> p t m", t=2),
                            hst_sb[:, kk, :, :],
                            start=(kk == 0),
                            stop=(kk == KK - 1),
                            perf_mode=perf_mode,
                        )
                    nc.vector.tensor_copy(ot[:, s5, :], ps[:, :J])
                    s += 1
                # Mid-stream stores ride SWDGE (Pool sequencer is idle) so a
                # store's sem-wait never head-of-line-blocks W loads queued
                # on sync/scalar; the final store (no W loads behind it)
                # uses HWDGE for its shorter fixed latency.
                seng = wq[g % 2] if g == len(groups) - 1 else nc.gpsimd
                seng.dma_start(
                    out[:, g0 * J : s * J].rearrange("p (s j) -> p s j", s=SG),
                    ot[:],
                )

    _split_excess_waits(nc, limit=1)
    _strip_const_memsets(nc)
    return nc


def pack_weights(Wq_core, swinterleave=USE_SWINTERLEAVE):
    """[VS, D] fp8 -> [P, NSUB, KK, 256] stationary image.

    DoubleRow:      block[t*128 + m] = Wq[s*128 + m,     kk*256 + t*128 + p]
    SwInterleave:   block[2*j + t]   = Wq[s*128 + 127-j, kk*256 + t*128 + p]
    """
    VS, D = Wq_core.shape
    KK = D // 256
    NSUB = VS // P
    A = np.ascontiguousarray(
        Wq_core.reshape(NSUB, P, KK, 2, P)
    )  # [s, m, kk, t, p]
    if swinterleave:
        Ar = A[:, ::-1]  # j = 127 - m
        out = Ar.transpose(4, 0, 2, 1, 3)  # [p, s, kk, j, t]
    else:
        out = A.transpose(4, 0, 2, 3, 1)  # [p, s, kk, t, m]
    return np.ascontiguousarray(out).reshape(P, NSUB, KK, 2 * P)


def pack_hidden(hs_sel_fp8):
    """[J, D] fp8 -> [P, KK, 2, J] with d = kk*256 + t*128 + p."""
    J, D = hs_sel_fp8.shape
    return np.ascontiguousarray(
        hs_sel_fp8.T.reshape(D // 256, 2, P, J).transpose(2, 0, 1, 3)
    )


def _job_indices(fill_tokens_num, num_generation_jobs):
    fill = np.asarray(fill_tokens_num, dtype=np.int64)
    fill_last = np.cumsum(fill) - 1
    total_fill = int(fill.sum())
    gen = total_fill + np.arange(int(num_generation_jobs), dtype=np.int64)
    return np.concatenate([fill_last, gen])


def kernel(hidden_states, embd_weight, fill_tokens_num, num_generation_jobs):
    hs = np.asarray(hidden_states, dtype=np.float32)
    W = np.asarray(embd_weight, dtype=np.float32)
    V, D = W.shape

    idx = _job_indices(fill_tokens_num, num_generation_jobs)
    J = idx.size

    hs_sel = hs[idx]  # [J, D] f32, kept for the exact rescore
    hst_host = pack_hidden(hs_sel.astype(ml_dtypes.float8_e4m3))

    # Device covers an even 128-aligned vocab shard per core; the short
    # tail (~2% of vocab) is scored exactly on the host (one dgemm, off
    # the device critical path).
    VS = (V // (N_CORES * P) - 1) * P  # per-core vocab shard width
    V_dev = VS * N_CORES
    Wq = (W[:V_dev] * W_SCALE).astype(ml_dtypes.float8_e4m3)
    in_maps = [
        {
            "hst": hst_host,
            "wt": pack_weights(Wq[i * VS : (i + 1) * VS]),
        }
        for i in range(N_CORES)
    ]

    nc = build_nc(D, J, VS)
    kernel.last_nc = nc
    kernel.last_in_maps = in_maps
    res = run_bass_kernel_spmd(nc, in_maps, core_ids=list(range(N_CORES)))
    kernel.last_results = res

    # out[p, s*J + j] = logits[s*128 + p, j]; concat cores over vocab.
    logits = np.concatenate(
        [
            res.results[i]["out"]
            .astype(np.float32)
            .reshape(P, VS // P, J)
            .transpose(1, 0, 2)
            .reshape(VS, J)
            for i in range(N_CORES)
        ],
        axis=0,
    ).T
    # Saturation note: max |scaled logit| here is ~287 > TRN e4m3's 240.
    # TRN clamps all overflow to its inf encoding, which ml_dtypes reads
    # as 256, so every saturated column ties the row max (m = 256) and
    # stays a candidate; the shortlist then keeps everything >= 192
    # scaled, and any true argmax in a row with saturation reads >= 220
    # (248 - fp8 err), so it is always shortlisted. Requires
    # DELTA >= (256 - 248) + 2*err ~ 35 scaled; DELTA = 64. NaN inputs
    # (true NaN only, not range overflow) map to +inf to stay candidates.
    logits = np.where(np.isnan(logits), np.inf, logits)

    # Columns within DELTA of each row's max, rescored exactly in f64.
    m = logits.max(axis=1, keepdims=True)
    rows, cols = np.nonzero(logits >= m - DELTA)
    exact = np.einsum(
        "ij,ij->i", hs_sel[rows].astype(np.float64), W[cols].astype(np.float64)
    )
    ids = np.zeros(J, dtype=np.int64)
    best = np.full(J, -np.inf)
    for r, c, s in zip(rows, cols, exact):
        if s > best[r]:
            best[r] = s
            ids[r] = c

    # Exact host logits for the vocab tail the device didn't cover.
    tail = hs_sel.astype(np.float64) @ W[V_dev:].astype(np.float64).T
    tail_best = tail.max(axis=1)
    tail_ids = V_dev + tail.argmax(axis=1)
    take = tail_best > best
    ids[take] = tail_ids[take]
    return ids.astype(np.int32)


# revision 34
# speedup vs baseline: 1.0081x; 1.0006x over previous
"""GreedySampler kernel for 8 Trainium2 NeuronCores.

The reference gathers 200 "last token" rows of hidden_states, computes
logits against the 50257x4096 embedding, and argmaxes over vocab
(softmax/log are monotonic). Cost is dominated by streaming the
embedding matrix: memory-bound. Tensor-parallel over vocab: each core
streams a 6144-column fp8 shard (25.2MB) once and computes fp8
DoubleRow logits; the host shortlists columns within DELTA of each
row's max and rescores them exactly in f64, so fp8 only nominates
candidates — it never decides the winner. The ~2% vocab tail left by
128-alignment is scored exactly on the host (one dgemm, off the device
critical path).

Device-side structure (vs the first working version, 96.3us):

  * W is pre-packed on the host into the exact SBUF image the PE wants,
    so every W load is a [128, 4KB-contiguous-per-partition] DMA
    (previously 512B gather lines) on the two HWDGE rings (sync/scalar,
    alternating). hst load + mid-stream logit stores ride SWDGE
    (gpsimd) so a store's sem-wait never head-of-line-blocks W loads;
    only the final store uses HWDGE for its shorter fixed latency.
  * Per-128-column vocab sub-block W DMAs start the PE after ~2us and
    give fine-grained load/compute overlap.
  * DoubleRowSwInterleave weight layout (host-interleaved) lets the HW
    read the stationary operand as one contiguous 256B/partition
    stream; the plain DoubleRow weight load (two non-contiguous
    128-column passes, ~+72% LDWEIGHTS time) was the likely critical
    path of the 96.3us version (800 x ~120ns = its entire runtime).
  * 40 warm-up matmuls on zeroed data during the DMA startup window
    hold PE activity so the HAM clock gate sits at 8/8 (2.4 GHz) when
    the first real matmul issues.
  * Logit stores batch 5 sub-blocks into [128, 1000B] lines (>=512B,
    no read-modify-write penalty) in a partition-major dram layout the
    host un-permutes; store groups taper at the end so the pipeline
    drains through a small final store.

Notes:
  * This walrus build rejects instructions carrying more than one sync
    wait, so after Tile scheduling we split excess waits onto nop
    instructions inserted just before the offender on the same engine
    queue (in-order execution keeps the semantics identical).
  * SwInterleave stationary contract (validated against HW): the
    flattened [p, 2, 128] AP view is pair-interleaved with reversed
    columns - block[2j + t] = W[col 127-j, k-row t].
"""

import numpy as np
import ml_dtypes

import concourse.bass as bass
import concourse.mybir as mybir
import concourse.tile as tile
from concourse.vector_clock import ScopedClock
from concourse.bass_utils import run_bass_kernel_spmd

P = 128
N_CORES = 8
W_SCALE = 32.0
DELTA = 2.0 * W_SCALE  # candidate margin in scaled-logit units

FP8 = mybir.dt.float8e4
F32 = mybir.dt.float32

USE_SWINTERLEAVE = True

_drain_patched = False


def _patch_tile_drain():
    """Split the tail Drain's sync waits (>1 rejected by this walrus)."""
    global _drain_patched
    if _drain_patched:
        return

    def _drain_and_barrier(self, tick_clock, wait_clock):
        nc = self.nc
        drain_inst = nc.sync.drain()
        wait_clock.add_sem_waits(
            drain_inst.ins, ScopedClock({None: tick_clock.global_clock})
        )
        si = drain_inst.ins.sync_info
        if si is not None and si.on_wait and len(si.on_wait) > 1:
            extra = list(si.on_wait[1:])
            del si.on_wait[1:]
            name2sem = {
                getattr(s, "name", None): s
                for s in self.sems.allocated().values()
            }
            for w in extra:
                nc.sync.wait_ge(name2sem[w.ant_name], w.wait_value)
        nc.all_engine_barrier()
        popped = nc._tile_sem_poison_stack.pop()
        assert popped is self._sem_poison
        nc.clear_and_free_semaphores(list(self.sems.allocated().values()))
        nc.all_engine_barrier()

    tile.TileContext._drain_and_barrier = _drain_and_barrier
    _drain_patched = True


def _split_excess_waits(nc, limit=1):
    """Move all but `limit` sync waits of every instruction onto nops
    inserted immediately before it on the same engine queue."""
    fn = nc.m.functions[0]
    for bb in fn.blocks:
        if not any(
            getattr(i, "sync_info", None) is not None
            and i.sync_info.on_wait
            and len(i.sync_info.on_wait) > limit
            for i in bb.instructions
        ):
            continue
        cur = nc.cur_bb.bb if hasattr(nc.cur_bb, "bb") else nc.cur_bb
        new_insts = []
        for inst in bb.instructions:
            si = getattr(inst, "sync_info", None)
            if si is not None and si.on_wait and len(si.on_wait) > limit:
                extra = list(si.on_wait[:-limit])
                del si.on_wait[: len(si.on_wait) - limit]
                for w in extra:
                    nop = nc.engines[inst.engine].nop(nofuse=True).ins
                    popped = cur.instructions.pop()  # nop() self-appended
                    assert popped is nop
                    nop.sync_info = mybir.SyncInfo(on_wait=[w], on_update=[])
                    new_insts.append(nop)
            new_insts.append(inst)
        bb.instructions[:] = new_insts
    return nc


def _strip_const_memsets(nc):
    """Drop the Bass-preamble memsets of the never-read const-* tensors
    (no readers in this kernel, no sync info) so the startup barrier —
    and with it the first W DMA — completes ~0.4us earlier."""
    for bb in nc.m.functions[0].blocks:
        keep = []
        for inst in bb.instructions:
            if type(inst).__name__ == "InstMemset" and "const-" in str(inst):
                si = getattr(inst, "sync_info", None)
                assert si is None or (not si.on_wait and not si.on_update)
                continue
            keep.append(inst)
        bb.instructions[:] = keep
    return nc


def build_nc(D, J, VS, store_group=None, swinterleave=USE_SWINTERLEAVE,
             w_bufs=16, ps_bufs=7, out_bufs=3, warmup_mms=56,
             split_tail_loads=1):
    """One core: logits for VS vocab columns x J jobs, fp8 in/out, fp32
    accumulation. W arrives pre-packed as [P, NSUB, KK, 256] where each
    256-byte block is the stationary operand for (sub, kk)."""
    _patch_tile_drain()
    KK = D // (2 * P)
    NSUB = VS // P
    if store_group is None:
        # Groups of 5 sub-blocks, tapering at the end so the final
        # stores are small and clear the pipeline quickly.
        groups, rem = [], NSUB
        while rem > 8:
            groups.append(5)
            rem -= 5
        groups += [rem - 4, 3, 1] if rem > 4 else [rem]
    elif isinstance(store_group, int):
        assert NSUB % store_group == 0
        groups = [store_group] * (NSUB // store_group)
    else:
        groups = list(store_group)
        assert sum(groups) == NSUB
    perf_mode = (
        mybir.MatmulPerfMode.DoubleRowSwInterleave
        if swinterleave
        else mybir.MatmulPerfMode.DoubleRow
    )

    nc = bass.Bass()
    hst = nc.dram_tensor("hst", [P, KK, 2, J], FP8, kind="ExternalInput")
    wt = nc.dram_tensor("wt", [P, NSUB, KK, 2 * P], FP8, kind="ExternalInput")
    # partition-major logits: out[p, s*J + j] = logits[s*128 + p, j]
    out = nc.dram_tensor("out", [P, NSUB * J], FP8, kind="ExternalOutput")

    with tile.TileContext(nc) as tc:
        with (
            tc.tile_pool(name="hs", bufs=1) as hs_pool,
            tc.tile_pool(name="w", bufs=w_bufs) as w_pool,
            tc.tile_pool(name="out", bufs=out_bufs) as out_pool,
            tc.tile_pool(name="ps", bufs=ps_bufs, space=bass.MemorySpace.PSUM) as ps_pool,
            tc.tile_pool(name="wu", bufs=1, space=bass.MemorySpace.PSUM) as wu_pool,
            tc.tile_pool(name="wub", bufs=1) as wub_pool,
        ):
            hst_sb = hs_pool.tile([P, KK, 2, J], FP8)
            # hst gates every matmul: load it via HWDGE (625ns gen) at the
            # head of the sync ring rather than SWDGE (~1us Q7 gen); the W
            # alternation starts on scalar so the rings stay balanced.
            nc.sync.dma_start(hst_sb[:], hst[:])

            if warmup_mms:
                # Dummy matmuls on zeroed data fill the DMA startup window
                # with PE activity so the HAM clock gate is already at
                # 8/8 (2.4 GHz) when the first real matmul issues.
                wu = wub_pool.tile([P, 16], FP8, name="wu")
                nc.vector.memset(wu[:], 0)
                wu_ps = wu_pool.tile([P, 512], F32, name="wu_ps")
                for _ in range(warmup_mms):
                    nc.tensor.matmul(
                        wu_ps[:8, :16],
                        wu[:, :8],
                        wu[:, :],
                        start=True,
                        stop=True,
                    )

            wq = [nc.sync, nc.scalar]
            s = 0
            for g, SG in enumerate(groups):
                g0 = s
                ot = out_pool.tile([P, SG, J], FP8, name="ot")
                for s5 in range(SG):
                    w_sb = w_pool.tile([P, KK, 2 * P], FP8, name="w_sb")
                    if s >= NSUB - split_tail_loads:
                        # The last loads gate the serial tail: split them
                        # so their matmuls start after half the line.
                        wq[(s + 1) % 2].dma_start(
                            w_sb[:, : KK // 2], wt[:, s, : KK // 2]
                        )
                        wq[(s + 1) % 2].dma_start(
                            w_sb[:, KK // 2 :], wt[:, s, KK // 2 :]
                        )
                    else:
                        wq[(s + 1) % 2].dma_start(w_sb[:], wt[:, s])
                    ps = ps_pool.tile([P, 512], F32, name="ps")
                    for kk in range(KK):
                        nc.tensor.matmul(
                            ps[:, :J],
                            w_sb[:, kk, :].rearrange("p (t m) -> p t m", t=2),
                            hst_sb[:, kk, :, :],
                            start=(kk == 0),
                            stop=(kk == KK - 1),
                            perf_mode=perf_mode,
                        )
                    nc.vector.tensor_copy(ot[:, s5, :], ps[:, :J])
                    s += 1
                # Mid-stream stores ride SWDGE (Pool sequencer is idle) so a
                # store's sem-wait never head-of-line-blocks W loads queued
                # on sync/scalar; the final store (no W loads behind it)
                # uses HWDGE for its shorter fixed latency.
                seng = wq[g % 2] if g == len(groups) - 1 else nc.gpsimd
                seng.dma_start(
                    out[:, g0 * J : s * J].rearrange("p (s j) -> p s j", s=SG),
                    ot[:],
                )

    _split_excess_waits(nc, limit=1)
    _strip_const_memsets(nc)
    return nc


def pack_weights(Wq_core, swinterleave=USE_SWINTERLEAVE):
    """[VS, D] fp8 -> [P, NSUB, KK, 256] stationary image.

    DoubleRow:      block[t*128 + m] = Wq[s*128 + m,     kk*256 + t*128 + p]
    SwInterleave:   block[2*j + t]   = Wq[s*128 + 127-j, kk*256 + t*128 + p]
    """
    VS, D = Wq_core.shape
    KK = D // 256
    NSUB = VS // P
    A = np.ascontiguousarray(
        Wq_core.reshape(NSUB, P, KK, 2, P)
    )  # [s, m, kk, t, p]
    if swinterleave:
        Ar = A[:, ::-1]  # j = 127 - m
        out = Ar.transpose(4, 0, 2, 1, 3)  # [p, s, kk, j, t]
    else:
        out = A.transpose(4, 0, 2, 3, 1)  # [p, s, kk, t, m]
    return np.ascontiguousarray(out).reshape(P, NSUB, KK, 2 * P)


def pack_hidden(hs_sel_fp8):
    """[J, D] fp8 -> [P, KK, 2, J] with d = kk*256 + t*128 + p."""
    J, D = hs_sel_fp8.shape
    return np.ascontiguousarray(
        hs_sel_fp8.T.reshape(D // 256, 2, P, J).transpose(2, 0, 1, 3)
    )


def _job_indices(fill_tokens_num, num_generation_jobs):
    fill = np.asarray(fill_tokens_num, dtype=np.int64)
    fill_last = np.cumsum(fill) - 1
    total_fill = int(fill.sum())
    gen = total_fill + np.arange(int(num_generation_jobs), dtype=np.int64)
    return np.concatenate([fill_last, gen])


def kernel(hidden_states, embd_weight, fill_tokens_num, num_generation_jobs):
    hs = np.asarray(hidden_states, dtype=np.float32)
    W = np.asarray(embd_weight, dtype=np.float32)
    V, D = W.shape

    idx = _job_indices(fill_tokens_num, num_generation_jobs)
    J = idx.size

    hs_sel = hs[idx]  # [J, D] f32, kept for the exact rescore
    hst_host = pack_hidden(hs_sel.astype(ml_dtypes.float8_e4m3))

    # Device covers an even 128-aligned vocab shard per core; the short
    # tail (~2% of vocab) is scored exactly on the host (one dgemm, off
    # the device critical path).
    VS = (V // (N_CORES * P) - 1) * P  # per-core vocab shard width
    V_dev = VS * N_CORES
    Wq = (W[:V_dev] * W_SCALE).astype(ml_dtypes.float8_e4m3)
    in_maps = [
        {
            "hst": hst_host,
            "wt": pack_weights(Wq[i * VS : (i + 1) * VS]),
        }
        for i in range(N_CORES)
    ]

    nc = build_nc(D, J, VS)
    kernel.last_nc = nc
    kernel.last_in_maps = in_maps
    res = run_bass_kernel_spmd(nc, in_maps, core_ids=list(range(N_CORES)))
    kernel.last_results = res

    # out[p, s*J + j] = logits[s*128 + p, j]; concat cores over vocab.
    logits = np.concatenate(
        [
            res.results[i]["out"]
            .astype(np.float32)
            .reshape(P, VS // P, J)
            .transpose(1, 0, 2)
            .reshape(VS, J)
            for i in range(N_CORES)
        ],
        axis=0,
    ).T
    # Saturation note: max |scaled logit| here is ~287 > TRN e4m3's 240.
    # TRN clamps all overflow to its inf encoding, which ml_dtypes reads
    # as 256, so every saturated column ties the row max (m = 256) and
    # stays a candidate; the shortlist then keeps everything >= 192
    # scaled, and any true argmax in a row with saturation reads >= 220
    # (248 - fp8 err), so it is always shortlisted. Requires
    # DELTA >= (256 - 248) + 2*err ~ 35 scaled; DELTA = 64. NaN inputs
    # (true NaN only, not range overflow) map to +inf to stay candidates.
    logits = np.where(np.isnan(logits), np.inf, logits)

    # Columns within DELTA of each row's max, rescored exactly in f64.
    m = logits.max(axis=1, keepdims=True)
    rows, cols = np.nonzero(logits >= m - DELTA)
    exact = np.einsum(
        "ij,ij->i", hs_sel[rows].astype(np.float64), W[cols].astype(np.float64)
    )
    ids = np.zeros(J, dtype=np.int64)
    best = np.full(J, -np.inf)
    for r, c, s in zip(rows, cols, exact):
        if s > best[r]:
            best[r] = s
            ids[r] = c

    # Exact host logits for the vocab tail the device didn't cover.
    tail = hs_sel.astype(np.float64) @ W[V_dev:].astype(np.float64).T
    tail_best = tail.max(axis=1)
    tail_ids = V_dev + tail.argmax(axis=1)
    take = tail_best > best
    ids[take] = tail_ids[take]
    return ids.astype(np.int32)
